# revision 21
# baseline (speedup 1.0000x reference)
"""Multi-head self-attention on 8 Trainium2 NeuronCores.

Tensor-parallel over heads: core c owns heads 2c, 2c+1 (128 of the 1024
hidden columns).  The host pre-transposes x to x^T [1024, 4096] bf16 AND
pre-arranges wq/wk/wv into [p=128, k=8, c=128] so every DMA is contiguous
(the old (k p) c -> p k c device-side rearrange issued 256B descriptor
lines at ~10GB/s and gated the prologue at ~25us).

Pipeline (per core):
  1. Q^T/K^T = (w.T @ x^T + b) in [d, token] layout (2 heads stacked on
     partitions: 0:64 head0, 64:128 head1).
  2. V^T likewise, then transposed 128x128-block-wise into V_aug
     [token, 65-per-head] (col 64/129 = 1.0 -> softmax denominator falls
     out of P@V).  The transposes are REGULAR matmuls against identity
     (lhsT=V^T block, rhs=I): transpose-mode runs at 1.2GHz (no HAM
     credit), a normal matmul runs warm at 2.4GHz.
  3. Attention in 4 chunks of (batch, 1024 queries), software-pipelined:
     scores^T tiles = K^T.T @ Q^T, P^T = exp(S^T/8) on ScalarE (the
     bottleneck engine: 128 exps of [128,1024] ~ 1.15us each = 147us),
     previous chunk's P@V + projection/WO back-work interleaved into the
     PE gaps.  Extras are balanced so no chunk oversubscribes the PE
     (the old schedule stuffed chunk 2 -> 6.6us ScalarE stall).
  4. partial = attnT.T @ wo[128 rows of this core] -> HBM (bf16).
     WO tiles 0-15 all run inside chunk 3's stream; the tail only owns
     PV(chunk3) + WO 16-31, interleaved as their attnT halves normalize.
Host sums the 8 partials and adds bo.

PSUM: 8 banks = scores pair (2x2) + PV accumulators (2) + extras (2).
Prologue: ~120 identity warmups keep the PE HAM-warm across the DMA
window so the first projections run at 2.4GHz.

Shapes hardcoded for x:[2,2048,1024], 16 heads, d_k=64.
"""

import numpy as np
import ml_dtypes

import concourse.bass as bass
import concourse.tile as tile
from concourse import bacc, mybir
from concourse.bass import ts
from concourse.bass_utils import run_bass_kernel_spmd

BF16 = mybir.dt.bfloat16
F32 = mybir.dt.float32
NPBF16 = ml_dtypes.bfloat16

B = 2
S = 2048
D = 1024
NT = B * S  # 4096 tokens
DK = 64
NCORES = 8
HPC = 2  # heads per core
SC = 1024  # attention s-chunk (exp op free size)

_CACHE = {}


def _build_nc():
    nc = bacc.Bacc("TRN2", target_bir_lowering=False, debug=False,
                   num_devices=NCORES)

    xT = nc.dram_tensor("xT", [D, NT], BF16, kind="ExternalInput").ap()
    wq = nc.dram_tensor("wq", [128, 8, 128], BF16, kind="ExternalInput").ap()
    wk = nc.dram_tensor("wk", [128, 8, 128], BF16, kind="ExternalInput").ap()
    wv = nc.dram_tensor("wv", [128, 8, 128], BF16, kind="ExternalInput").ap()
    bqkv = nc.dram_tensor("bqkv", [128, 3], F32, kind="ExternalInput").ap()
    wo = nc.dram_tensor("wo", [128, D], BF16, kind="ExternalInput").ap()
    out = nc.dram_tensor("out", [NT, D], BF16, kind="ExternalOutput").ap()

    with tile.TileContext(nc) as tc:
        _emit(nc, tc, xT, wq, wk, wv, bqkv, wo, out)
    nc.compile()
    return nc


def _emit(nc, tc, xT, wq, wk, wv, bqkv, wo, out):
    import contextlib
    ctx = contextlib.ExitStack()
    with ctx:
        consts = ctx.enter_context(tc.tile_pool(name="consts", bufs=1))
        ptp = ctx.enter_context(tc.tile_pool(name="ptp", bufs=44))
        psp = ctx.enter_context(tc.tile_pool(name="psp", bufs=2, space="PSUM"))
        pvp = ctx.enter_context(tc.tile_pool(name="pvp", bufs=2, space="PSUM"))
        pse = ctx.enter_context(tc.tile_pool(name="pse", bufs=2, space="PSUM"))
        stg = ctx.enter_context(tc.tile_pool(name="stg", bufs=2))
        nrm = ctx.enter_context(tc.tile_pool(name="nrm", bufs=1))

        # ---- persistent SBUF tensors ----
        xT_sb = consts.tile([128, 8, NT], BF16)      # 8 k-tiles of x^T
        wq_sb = consts.tile([128, 8, 128], BF16)
        wk_sb = consts.tile([128, 8, 128], BF16)
        wv_sb = consts.tile([128, 8, 128], BF16)
        bqkv_sb = consts.tile([128, 3], F32)
        bq_sb = bqkv_sb[:, 0:1]
        bk_sb = bqkv_sb[:, 1:2]
        bv_sb = bqkv_sb[:, 2:3]
        wo_sb = consts.tile([128, D], BF16)
        QT = consts.tile([128, NT], BF16)
        KT = consts.tile([128, NT], BF16)
        V_sb = consts.tile([128, 32, 130], BF16)     # [t-in-tile, t_tile, col]
        attnT = consts.tile([128, NT], BF16)
        ident = consts.tile([128, 128], BF16)
        vstg = ctx.enter_context(tc.tile_pool(name="vstg", bufs=2))

        xT_d = xT.rearrange("(k p) n -> k p n", p=128)
        # x^T lands in first-use order on the sync/scalar DMA queues
        # (alternating by k so the two queues split each column band).
        def xdma(k, lo, hi):
            eng = nc.sync if k % 2 == 0 else nc.scalar
            eng.dma_start(out=xT_sb[:, k, lo:hi], in_=xT_d[k][:, lo:hi])
        for k in range(8):
            xdma(k, 0, 512)
        for k in range(8):
            xdma(k, 512, 1024)
        # identity (gpsimd) first so the PE warm-up can start immediately
        from concourse.masks import make_identity
        make_identity(nc, ident)
        # weights + biases on the gpsimd queue: all contiguous (host
        # pre-arranged [p, k, c]), away from the 16MB of x/out traffic.
        # Biases are one packed [128,3] transfer — three separate [128,1]
        # tensors cost ~4.5us of 4B-line descriptors and head-of-line
        # block the weights.
        nc.gpsimd.dma_start(out=bqkv_sb, in_=bqkv)
        nc.gpsimd.dma_start(out=wk_sb, in_=wk)
        nc.gpsimd.dma_start(out=wq_sb, in_=wq)
        nc.gpsimd.dma_start(out=wv_sb, in_=wv)
        nc.gpsimd.dma_start(out=wo_sb, in_=wo)
        # PE warm-up: identity matmuls bridge the DMA window so the HAM
        # clock-gate stays at 8/8 when the real projections start
        wups = pse.tile([128, 128], F32, tag="pse", name="wups")
        for i in range(180):
            nc.tensor.matmul(wups, lhsT=ident, rhs=ident, start=True, stop=True)
        for k in range(8):
            xdma(k, 1024, 2048)

        # trigger the exp ACT-table load early (~2.7us) while DMAs run
        tblw = stg.tile([128, 1], F32, tag="ob")
        nc.scalar.activation(out=tblw, in_=bq_sb,
                             func=mybir.ActivationFunctionType.Exp)

        # ones columns of V_aug (never touched by the per-tile copies)
        nc.vector.memset(V_sb[:, :, 64:65], 1.0)
        nc.vector.memset(V_sb[:, :, 129:130], 1.0)
        for k in range(8):
            xdma(k, S, NT)

        # ---- emit helpers (psum shared with the scores tag) ----
        vt_stage = {}

        def emit_v_proj(c):
            # V^T chunk: [c128, 512 tokens] += wv[k].T @ xT[k] (+bias, ->bf16)
            psv = pse.tile([128, 512], F32, tag="pse")
            for k in range(8):
                nc.tensor.matmul(psv, lhsT=wv_sb[:, k, :],
                                 rhs=xT_sb[:, k, ts(c, 512)],
                                 start=(k == 0), stop=(k == 7))
            vt = vstg.tile([128, 512], BF16, tag="vt", name=f"vt{c}")
            nc.vector.tensor_scalar_add(vt, psv, bv_sb)
            vt_stage[c] = vt

        def emit_v_tr(tt):
            # transpose one 128x128 block of V^T into V_aug [t, col] layout.
            # Regular matmul against identity (not transpose-mode): runs at
            # the warm 2.4GHz clock and keeps HAM credit.
            c, j = divmod(tt, 4)
            trp = pse.tile([128, 128], F32, tag="pse", name=f"trp{tt}")
            nc.tensor.matmul(trp, lhsT=vt_stage[c][:, ts(j, 128)], rhs=ident,
                             start=True, stop=True)
            nc.vector.tensor_copy(V_sb[:, tt, 0:64], trp[:, 0:64])
            nc.vector.tensor_copy(V_sb[:, tt, 65:129], trp[:, 64:128])

        def emit_wo_tile(tt, use_act=False):
            # one [128 tokens, 1024] output tile: 2 matmul halves into one
            # ob staging tile, a single 256KB DMA (out triggers alternate
            # sync/gpsimd so the store stream shares two hardware queues)
            ob = stg.tile([128, 1024], BF16, tag="ob")
            for eh in range(2):
                pw = pse.tile([128, 512], F32, tag="pse")
                nc.tensor.matmul(pw, lhsT=attnT[:, ts(tt, 128)],
                                 rhs=wo_sb[:, ts(eh, 512)],
                                 start=True, stop=True)
                if use_act and eh == 1:
                    nc.scalar.activation(
                        out=ob[:, ts(eh, 512)], in_=pw,
                        func=mybir.ActivationFunctionType.Copy, bias=0.0)
                else:
                    nc.vector.tensor_copy(ob[:, ts(eh, 512)], pw)
            nc.sync.dma_start(out=out[tt * 128:(tt + 1) * 128, :], in_=ob)

        def emit_proj_chunk(w_sb, b_sb, o_sb, n, w=512):
            # w-token chunk n (units of w) of the Q^T or K^T projection
            ps = pse.tile([128, 512], F32, tag="pse")
            for k in range(8):
                nc.tensor.matmul(ps[:, 0:w], lhsT=w_sb[:, k, :],
                                 rhs=xT_sb[:, k, ts(n, w)],
                                 start=(k == 0), stop=(k == 7))
            nc.vector.tensor_scalar_add(o_sb[:, ts(n, w)], ps[:, 0:w], b_sb)

        def emit_normalize_half(prev, c):
            # the h0/h1 chains are interleaved (per-h tags) so the two
            # gpsimd broadcasts queue back-to-back instead of serializing
            # behind each other's DVE work (~3.5us instead of ~7us)
            b, sc, pts, pv_state = prev
            s0 = b * S + sc * SC + c * 512
            psos = pv_state['psos']
            for h in range(HPC):
                pso = psos[h]
                # stage the accumulator out of PSUM first so the bank frees
                # for the next PV half after one copy, not the whole
                # recip/broadcast chain
                ostg = nrm.tile([65, 512], F32, tag="ostg", name=f"ostg{h}")
                nc.vector.tensor_copy(ostg, pso[0:65, :])
                rsum = nrm.tile([1, 512], F32, tag="rsum")
                nc.vector.tensor_copy(rsum, ostg[64:65, :])
                nc.vector.reciprocal_approx_fast(out=rsum, in_=rsum)
                recb = nrm.tile([64, 512], F32, tag="recb")
                nc.gpsimd.partition_broadcast(recb, rsum)
                nc.vector.tensor_mul(
                    attnT[h * DK:(h + 1) * DK, s0:s0 + 512],
                    ostg[0:64, :], recb)

        def emit_pv_step(prev, s):
            # one pipeline step of P@V for the previous chunk: two t-tiles
            # into the [65, 512] accumulators of half-chunk c = s // 8
            b, sc, pts, pv_state = prev
            c = s // 8
            if s % 8 == 0:
                pv_state['psos'] = [
                    pvp.tile([128, 512], F32, tag="pv",
                             name=f"pso{b}_{sc}_{c}_{h}")
                    for h in range(HPC)]
            psos = pv_state['psos']
            for dt in range(2):
                tt = 2 * (s % 8) + dt
                for h in range(HPC):
                    nc.tensor.matmul(
                        psos[h][0:65, :],
                        lhsT=V_sb[:, b * 16 + tt, h * 65:(h + 1) * 65],
                        rhs=pts[tt][h][:, ts(c, 512)],
                        start=(tt == 0), stop=(tt == 15))
            if s % 8 == 7:
                emit_normalize_half(prev, c)

        # ---- minimal prologue: scores consume K^T at only 128 cols per
        # step, so just KT[:, 0:256] is built up front (the rest arrives as
        # in-loop extras); QT 0:1024 accumulates k-interleaved chasing the
        # x^T DMAs — its arrival is the only gate on the first exp.
        psKm = pse.tile([128, 256], F32, tag="pse", name="psKm")
        for k in range(8):
            nc.tensor.matmul(psKm, lhsT=wk_sb[:, k, :],
                             rhs=xT_sb[:, k, 0:256],
                             start=(k == 0), stop=(k == 7))
        nc.vector.tensor_scalar_add(KT[:, 0:256], psKm, bk_sb)
        psQ0 = pse.tile([128, 512], F32, tag="pse", name="psQ0")
        psQ1 = pse.tile([128, 512], F32, tag="pse", name="psQ1")
        for k in range(8):
            nc.tensor.matmul(psQ0, lhsT=wq_sb[:, k, :],
                             rhs=xT_sb[:, k, 0:512],
                             start=(k == 0), stop=(k == 7))
            nc.tensor.matmul(psQ1, lhsT=wq_sb[:, k, :],
                             rhs=xT_sb[:, k, 512:1024],
                             start=(k == 0), stop=(k == 7))
        nc.vector.tensor_scalar_add(QT[:, 0:512], psQ0, bq_sb)
        nc.vector.tensor_scalar_add(QT[:, 512:1024], psQ1, bq_sb)

        # deferred PE work, interleaved into the ACT-bound attention loop.
        # entry = (step, thunk): emitted at the given tt step of that chunk.
        # Budget: each chunk step has ~1.0us of PE slack beside its own
        # scores pair + the previous chunk's PV; chunk 0 has no PV so it
        # absorbs the V-batch0 build and most of the KT back-fill.
        qk = [(wq_sb, bq_sb, QT), (wk_sb, bk_sb, KT)]

        def pj(which, n256):
            return lambda: emit_proj_chunk(*qk[which], n256, w=256)

        def spread(thunks, start, stop):
            n = len(thunks)
            return [(start + (i * (stop - start)) // n, t)
                    for i, t in enumerate(thunks)]

        def v_ex(cs, start, stop):
            # V proj chunks cs (512-token units) + their transposes
            ex = []
            n = len(cs)
            for i, c in enumerate(cs):
                st = start + (i * (stop - start)) // n
                ex.append((st, lambda c=c: emit_v_proj(c)))
                ex += [(st + 1 + j, lambda t=4 * c + j: emit_v_tr(t))
                       for j in range(4)]
            return ex

        extras_per_chunk = [
            # chunk 0 (b0,sc0; no PV, ~23us slack): KT units 1-10 just
            # ahead of use (unit u feeds scores step 2u), QT 4-7 (chunk 1
            # queries), V batch0 (tiles 0-15, needed by chunk 1's PV) and
            # the first batch1 V chunk (c4: its x arrives by ~45us)
            [(0, pj(1, 1)), (1, pj(1, 2)), (2, pj(1, 3)), (3, pj(1, 4)),
             (5, pj(1, 5)), (7, pj(1, 6)), (9, pj(1, 7)), (11, pj(1, 8)),
             (13, pj(1, 9)), (14, pj(1, 10))]
            + spread([pj(0, n) for n in (4, 5, 6, 7)], 8, 13)
            + v_ex([0, 1, 2], 1, 7) + v_ex([3, 4], 7, 12),
            # chunk 1 (b0,sc1): QT 8-11 (chunk 2 queries), KT 11-13, V c5
            spread([pj(0, n) for n in (8, 9, 10, 11)], 0, 8)
            + [(8, pj(1, 11)), (10, pj(1, 12)), (12, pj(1, 13))]
            + v_ex([5], 4, 9),
            # chunk 2 (b1,sc0): KT 14-15 (feed scores steps 12/14), QT
            # 12-15 (chunk 3 queries), V batch1 rest (tiles 24-31)
            [(0, pj(1, 14)), (2, pj(1, 15))]
            + spread([pj(0, n) for n in (12, 13, 14, 15)], 3, 10)
            + v_ex([6], 5, 10) + v_ex([7], 9, 14),
            # chunk 3 (b1,sc1): WO tiles 0-15 (tokens of chunks 0-1, all
            # normalized by the end of chunk 2)
            [((i * 14) // 16, lambda t=t: emit_wo_tile(t))
             for i, t in enumerate(range(16))],
        ]

        def emit_scores(b, sc, tt):
            # one t-tile of S^T for both heads -> psum pair; returns the pair
            s0 = b * S + sc * SC
            pair = []
            for h in range(HPC):
                ps = psp.tile([128, SC], F32, tag="ps")
                hsl = slice(h * DK, (h + 1) * DK)
                for n2 in range(SC // 512):
                    nc.tensor.matmul(
                        ps[:, ts(n2, 512)],
                        lhsT=KT[hsl, b * S + tt * 128:b * S + (tt + 1) * 128],
                        rhs=QT[hsl, s0 + n2 * 512:s0 + (n2 + 1) * 512],
                        start=True, stop=True)
                pair.append(ps)
            return pair

        chunks = [(b, sc) for b in range(B) for sc in range(S // SC)]
        prev = None
        next_pair = None
        for ci, (b, sc) in enumerate(chunks):
            extras = sorted(extras_per_chunk[ci], key=lambda e: e[0])
            pts = []
            cur = (b, sc, pts, {})
            pair = next_pair if next_pair is not None else emit_scores(b, sc, 0)
            for tt in range(16):
                row = []
                for h in range(HPC):
                    pt = ptp.tile([128, SC], BF16, tag="pt")
                    nc.scalar.activation(
                        out=pt, in_=pair[h],
                        func=mybir.ActivationFunctionType.Exp,
                        scale=0.125)
                    row.append(pt)
                pts.append(row)
                # emit next scores ahead of the slower PE work so ScalarE's
                # psum slots refill as soon as its exp frees them
                if tt + 1 < 16:
                    pair = emit_scores(b, sc, tt + 1)
                if prev is not None:
                    emit_pv_step(prev, tt)
                while extras and extras[0][0] <= tt:
                    extras.pop(0)[1]()
            # next chunk's first scores go ahead of the extras flush so the
            # boundary never starves ScalarE
            if ci + 1 < len(chunks):
                nb, nsc = chunks[ci + 1]
                next_pair = emit_scores(nb, nsc, 0)
            for _, e in extras:
                e()
            prev = cur
        # tail: PV + normalize for the last chunk with WO 16-23 (ready at
        # tail start) at odd steps.  Tail-only WO tiles route their psum
        # through the now-idle scores pool (psp: 2x2 banks) and drain with
        # ONE [128,1024] copy, alternating ACT/DVE per tile, so the copies
        # don't gate the next matmul through the 2-slot pse ring.
        def emit_wo_tile_tail(tt, use_act):
            ob = stg.tile([128, 1024], BF16, tag="ob")
            pw = psp.tile([128, SC], F32, tag="ps")
            for eh in range(2):
                nc.tensor.matmul(pw[:, ts(eh, 512)],
                                 lhsT=attnT[:, ts(tt, 128)],
                                 rhs=wo_sb[:, ts(eh, 512)],
                                 start=True, stop=True)
            if use_act:
                nc.scalar.activation(
                    out=ob, in_=pw,
                    func=mybir.ActivationFunctionType.Copy, bias=0.0)
            else:
                nc.vector.tensor_copy(ob, pw)
            nc.sync.dma_start(out=out[tt * 128:(tt + 1) * 128, :], in_=ob)

        # WO 24-27 depend only on the tail's half-0 normalize (emitted at
        # step 7, ~2.5us of DVE+gpsimd): by step 12 it's long done, so
        # these interleave into the late PV steps without stalling the
        # in-order PE queue.
        for s in range(16):
            emit_pv_step(prev, s)
            if s % 2 == 1:
                emit_wo_tile(16 + s // 2, use_act=True)
            if s >= 12:
                emit_wo_tile_tail(24 + (s - 12), use_act=(s % 2 == 0))
        for i, tt in enumerate(range(28, 32)):
            emit_wo_tile_tail(tt, use_act=(i % 2 == 0))


def _prep_in_maps(x, wq, bq, wk, bk, wv, bv, wo):
    x2 = np.asarray(x, np.float32).reshape(NT, D)
    xT = np.ascontiguousarray(x2.T).astype(NPBF16)
    wq = np.asarray(wq, np.float32)
    wk = np.asarray(wk, np.float32)
    wv = np.asarray(wv, np.float32)
    wo = np.asarray(wo, np.float32)
    bq = np.asarray(bq, np.float32)
    bk = np.asarray(bk, np.float32)
    bv = np.asarray(bv, np.float32)

    def pkc(w, cs):
        # [1024, 128-cols-of-core] -> [p=128, k=8, c=128] contiguous
        wc = w[:, cs].reshape(8, 128, 128).transpose(1, 0, 2)
        return np.ascontiguousarray(wc).astype(NPBF16)

    in_maps = []
    for c in range(NCORES):
        cs = slice(c * 128, (c + 1) * 128)
        in_maps.append({
            "xT": xT,
            "wq": pkc(wq, cs),
            "wk": pkc(wk, cs),
            "wv": pkc(wv, cs),
            "bqkv": np.ascontiguousarray(
                np.stack([bq[cs], bk[cs], bv[cs]], axis=1)),
            "wo": wo[cs, :].astype(NPBF16),
        })
    return in_maps


def kernel(x, wq, bq, wk, bk, wv, bv, wo, bo, _run_kwargs=None):
    if "nc" not in _CACHE:
        _CACHE["nc"] = _build_nc()
    nc = _CACHE["nc"]
    in_maps = _prep_in_maps(x, wq, bq, wk, bk, wv, bv, wo)
    res = run_bass_kernel_spmd(nc, in_maps, list(range(NCORES)),
                               **(_run_kwargs or {}))
    acc = np.zeros((NT, D), np.float32)
    for c in range(NCORES):
        acc += res.results[c]["out"].astype(np.float32)
    acc += np.asarray(bo, np.float32)[None, :]
    if _run_kwargs:
        _CACHE["last_results"] = res
    return acc.reshape(B, S, D)


# revision 24
# speedup vs baseline: 1.0218x; 1.0218x over previous
"""Multi-head self-attention on 8 Trainium2 NeuronCores.

Tensor-parallel over heads: core c owns heads 2c, 2c+1 (128 of the 1024
hidden columns).  The host pre-transposes x to x^T [1024, 4096] bf16 AND
pre-arranges wq/wk/wv into [p=128, k=8, c=128] so every DMA is contiguous
(the old (k p) c -> p k c device-side rearrange issued 256B descriptor
lines at ~10GB/s and gated the prologue at ~25us).

Pipeline (per core):
  1. Q^T/K^T = (w.T @ x^T + b) in [d, token] layout (2 heads stacked on
     partitions: 0:64 head0, 64:128 head1).
  2. V^T likewise, then transposed 128x128-block-wise into V_aug
     [token, 65-per-head] (col 64/129 = 1.0 -> softmax denominator falls
     out of P@V).  The transposes are REGULAR matmuls against identity
     (lhsT=V^T block, rhs=I): transpose-mode runs at 1.2GHz (no HAM
     credit), a normal matmul runs warm at 2.4GHz.
  3. Attention in 4 chunks of (batch, 1024 queries), software-pipelined:
     scores^T tiles = K^T.T @ Q^T, P^T = exp(S^T/8) on ScalarE (the
     bottleneck engine: 128 exps of [128,1024] ~ 1.15us each = 147us),
     previous chunk's P@V + projection/WO back-work interleaved into the
     PE gaps.  Extras are balanced so no chunk oversubscribes the PE
     (the old schedule stuffed chunk 2 -> 6.6us ScalarE stall).
  4. partial = attnT.T @ wo[128 rows of this core] -> HBM (bf16).
     WO tiles 0-15 all run inside chunk 3's stream; the tail only owns
     PV(chunk3) + WO 16-31, interleaved as their attnT halves normalize.
Host sums the 8 partials and adds bo.

PSUM: 8 banks = scores pair (2x2) + PV accumulators (2) + extras (2).
Prologue: ~120 identity warmups keep the PE HAM-warm across the DMA
window so the first projections run at 2.4GHz.

Shapes hardcoded for x:[2,2048,1024], 16 heads, d_k=64.
"""

import numpy as np
import ml_dtypes

import concourse.bass as bass
import concourse.tile as tile
from concourse import bacc, mybir
from concourse.bass import ts
from concourse.bass_utils import run_bass_kernel_spmd

BF16 = mybir.dt.bfloat16
F32 = mybir.dt.float32
NPBF16 = ml_dtypes.bfloat16

B = 2
S = 2048
D = 1024
NT = B * S  # 4096 tokens
DK = 64
NCORES = 8
HPC = 2  # heads per core
SC = 1024  # attention s-chunk (exp op free size)

_CACHE = {}


def _build_nc():
    nc = bacc.Bacc("TRN2", target_bir_lowering=False, debug=False,
                   num_devices=NCORES)

    xT = nc.dram_tensor("xT", [D, NT], BF16, kind="ExternalInput").ap()
    wq = nc.dram_tensor("wq", [128, 8, 128], BF16, kind="ExternalInput").ap()
    wk = nc.dram_tensor("wk", [128, 8, 128], BF16, kind="ExternalInput").ap()
    wv = nc.dram_tensor("wv", [128, 8, 128], BF16, kind="ExternalInput").ap()
    bqkv = nc.dram_tensor("bqkv", [128, 3], F32, kind="ExternalInput").ap()
    wo = nc.dram_tensor("wo", [128, D], BF16, kind="ExternalInput").ap()
    out = nc.dram_tensor("out", [NT, D], BF16, kind="ExternalOutput").ap()

    with tile.TileContext(nc) as tc:
        _emit(nc, tc, xT, wq, wk, wv, bqkv, wo, out)
    nc.compile()
    return nc


def _emit(nc, tc, xT, wq, wk, wv, bqkv, wo, out):
    import contextlib
    ctx = contextlib.ExitStack()
    with ctx:
        consts = ctx.enter_context(tc.tile_pool(name="consts", bufs=1))
        ptp = ctx.enter_context(tc.tile_pool(name="ptp", bufs=44))
        psp = ctx.enter_context(tc.tile_pool(name="psp", bufs=2, space="PSUM"))
        pvp = ctx.enter_context(tc.tile_pool(name="pvp", bufs=2, space="PSUM"))
        pse = ctx.enter_context(tc.tile_pool(name="pse", bufs=2, space="PSUM"))
        stg = ctx.enter_context(tc.tile_pool(name="stg", bufs=2))
        nrm = ctx.enter_context(tc.tile_pool(name="nrm", bufs=1))

        # ---- persistent SBUF tensors ----
        xT_sb = consts.tile([128, 8, NT], BF16)      # 8 k-tiles of x^T
        wq_sb = consts.tile([128, 8, 128], BF16)
        wk_sb = consts.tile([128, 8, 128], BF16)
        wv_sb = consts.tile([128, 8, 128], BF16)
        bqkv_sb = consts.tile([128, 3], F32)
        bq_sb = bqkv_sb[:, 0:1]
        bk_sb = bqkv_sb[:, 1:2]
        bv_sb = bqkv_sb[:, 2:3]
        wo_sb = consts.tile([128, D], BF16)
        QT = consts.tile([128, NT], BF16)
        KT = consts.tile([128, NT], BF16)
        V_sb = consts.tile([128, 32, 130], BF16)     # [t-in-tile, t_tile, col]
        attnT = consts.tile([128, NT], BF16)
        ident = consts.tile([128, 128], BF16)
        vstg = ctx.enter_context(tc.tile_pool(name="vstg", bufs=2))

        xT_d = xT.rearrange("(k p) n -> k p n", p=128)
        # x^T lands in first-use order on the sync/scalar DMA queues
        # (alternating by k so the two queues split each column band).
        def xdma(k, lo, hi, late=False):
            # late batches trigger from the sync engine only: a trigger
            # instruction costs ~600ns of engine time, and on the scalar
            # engine the 1024:4096 batches were executing at 22-28us —
            # exactly when ScalarE should stream its first exps
            eng = nc.sync if (late or k % 2 == 0) else nc.scalar
            eng.dma_start(out=xT_sb[:, k, lo:hi], in_=xT_d[k][:, lo:hi])
        for k in range(8):
            xdma(k, 0, 512)
        for k in range(8):
            xdma(k, 512, 1024)
        # identity (gpsimd) first so the PE warm-up can start immediately
        from concourse.masks import make_identity
        make_identity(nc, ident)
        # weights + biases on the gpsimd queue: all contiguous (host
        # pre-arranged [p, k, c]), away from the 16MB of x/out traffic.
        # Biases are one packed [128,3] transfer — three separate [128,1]
        # tensors cost ~4.5us of 4B-line descriptors and head-of-line
        # block the weights.
        nc.gpsimd.dma_start(out=bqkv_sb, in_=bqkv)
        nc.gpsimd.dma_start(out=wk_sb, in_=wk)
        nc.gpsimd.dma_start(out=wq_sb, in_=wq)
        nc.gpsimd.dma_start(out=wv_sb, in_=wv)
        nc.gpsimd.dma_start(out=wo_sb, in_=wo)
        # PE warm-up: identity matmuls bridge the DMA window so the HAM
        # clock-gate stays at 8/8 when the real projections start
        wups = pse.tile([128, 128], F32, tag="pse", name="wups")
        for i in range(130):
            nc.tensor.matmul(wups, lhsT=ident, rhs=ident, start=True, stop=True)
        for k in range(8):
            xdma(k, 1024, 2048, late=True)

        # trigger the exp ACT-table load early (~2.7us) while DMAs run
        tblw = stg.tile([128, 1], F32, tag="ob")
        nc.scalar.activation(out=tblw, in_=bq_sb,
                             func=mybir.ActivationFunctionType.Exp)

        # ones columns of V_aug (never touched by the per-tile copies)
        nc.vector.memset(V_sb[:, :, 64:65], 1.0)
        nc.vector.memset(V_sb[:, :, 129:130], 1.0)
        for k in range(8):
            xdma(k, S, NT, late=True)

        # ---- emit helpers (psum shared with the scores tag) ----
        vt_stage = {}

        def emit_v_proj(c):
            # V^T chunk: [c128, 512 tokens] += wv[k].T @ xT[k] (+bias, ->bf16)
            psv = pse.tile([128, 512], F32, tag="pse")
            for k in range(8):
                nc.tensor.matmul(psv, lhsT=wv_sb[:, k, :],
                                 rhs=xT_sb[:, k, ts(c, 512)],
                                 start=(k == 0), stop=(k == 7))
            vt = vstg.tile([128, 512], BF16, tag="vt", name=f"vt{c}")
            nc.vector.tensor_scalar_add(vt, psv, bv_sb)
            vt_stage[c] = vt

        def emit_v_tr(tt):
            # transpose one 128x128 block of V^T into V_aug [t, col] layout.
            # Regular matmul against identity (not transpose-mode): runs at
            # the warm 2.4GHz clock and keeps HAM credit.
            c, j = divmod(tt, 4)
            trp = pse.tile([128, 128], F32, tag="pse", name=f"trp{tt}")
            nc.tensor.matmul(trp, lhsT=vt_stage[c][:, ts(j, 128)], rhs=ident,
                             start=True, stop=True)
            nc.vector.tensor_copy(V_sb[:, tt, 0:64], trp[:, 0:64])
            nc.vector.tensor_copy(V_sb[:, tt, 65:129], trp[:, 64:128])

        def emit_wo_tile(tt, use_act=False):
            # one [128 tokens, 1024] output tile: 2 matmul halves into one
            # ob staging tile, a single 256KB DMA (out triggers alternate
            # sync/gpsimd so the store stream shares two hardware queues)
            ob = stg.tile([128, 1024], BF16, tag="ob")
            for eh in range(2):
                pw = pse.tile([128, 512], F32, tag="pse")
                nc.tensor.matmul(pw, lhsT=attnT[:, ts(tt, 128)],
                                 rhs=wo_sb[:, ts(eh, 512)],
                                 start=True, stop=True)
                if use_act and eh == 1:
                    nc.scalar.activation(
                        out=ob[:, ts(eh, 512)], in_=pw,
                        func=mybir.ActivationFunctionType.Copy, bias=0.0)
                else:
                    nc.vector.tensor_copy(ob[:, ts(eh, 512)], pw)
            nc.sync.dma_start(out=out[tt * 128:(tt + 1) * 128, :], in_=ob)

        def emit_proj_chunk(w_sb, b_sb, o_sb, n, w=512):
            # w-token chunk n (units of w) of the Q^T or K^T projection
            ps = pse.tile([128, 512], F32, tag="pse")
            for k in range(8):
                nc.tensor.matmul(ps[:, 0:w], lhsT=w_sb[:, k, :],
                                 rhs=xT_sb[:, k, ts(n, w)],
                                 start=(k == 0), stop=(k == 7))
            nc.vector.tensor_scalar_add(o_sb[:, ts(n, w)], ps[:, 0:w], b_sb)

        def emit_normalize_half(prev, c):
            # the h0/h1 chains are interleaved (per-h tags) so the two
            # gpsimd broadcasts queue back-to-back instead of serializing
            # behind each other's DVE work (~3.5us instead of ~7us)
            b, sc, pts, pv_state = prev
            s0 = b * S + sc * SC + c * 512
            psos = pv_state['psos']
            for h in range(HPC):
                pso = psos[h]
                # stage the accumulator out of PSUM first so the bank frees
                # for the next PV half after one copy, not the whole
                # recip/broadcast chain
                ostg = nrm.tile([65, 512], F32, tag="ostg", name=f"ostg{h}")
                nc.vector.tensor_copy(ostg, pso[0:65, :])
                rsum = nrm.tile([1, 512], F32, tag="rsum")
                nc.vector.tensor_copy(rsum, ostg[64:65, :])
                nc.vector.reciprocal_approx_fast(out=rsum, in_=rsum)
                recb = nrm.tile([64, 512], F32, tag="recb")
                nc.gpsimd.partition_broadcast(recb, rsum)
                nc.vector.tensor_mul(
                    attnT[h * DK:(h + 1) * DK, s0:s0 + 512],
                    ostg[0:64, :], recb)

        def emit_pv_step(prev, s):
            # one pipeline step of P@V for the previous chunk: two t-tiles
            # into the [65, 512] accumulators of half-chunk c = s // 8
            b, sc, pts, pv_state = prev
            c = s // 8
            if s % 8 == 0:
                pv_state['psos'] = [
                    pvp.tile([128, 512], F32, tag="pv",
                             name=f"pso{b}_{sc}_{c}_{h}")
                    for h in range(HPC)]
            psos = pv_state['psos']
            for dt in range(2):
                tt = 2 * (s % 8) + dt
                for h in range(HPC):
                    nc.tensor.matmul(
                        psos[h][0:65, :],
                        lhsT=V_sb[:, b * 16 + tt, h * 65:(h + 1) * 65],
                        rhs=pts[tt][h][:, ts(c, 512)],
                        start=(tt == 0), stop=(tt == 15))
            if s % 8 == 7:
                emit_normalize_half(prev, c)

        # ---- minimal prologue: scores consume K^T at only 128 cols per
        # step, so just KT[:, 0:256] is built up front (the rest arrives as
        # in-loop extras); QT 0:1024 accumulates k-interleaved chasing the
        # x^T DMAs — its arrival is the only gate on the first exp.
        psKm = pse.tile([128, 256], F32, tag="pse", name="psKm")
        for k in range(8):
            nc.tensor.matmul(psKm, lhsT=wk_sb[:, k, :],
                             rhs=xT_sb[:, k, 0:256],
                             start=(k == 0), stop=(k == 7))
        nc.vector.tensor_scalar_add(KT[:, 0:256], psKm, bk_sb)
        psQ0 = pse.tile([128, 512], F32, tag="pse", name="psQ0")
        psQ1 = pse.tile([128, 512], F32, tag="pse", name="psQ1")
        for k in range(8):
            nc.tensor.matmul(psQ0, lhsT=wq_sb[:, k, :],
                             rhs=xT_sb[:, k, 0:512],
                             start=(k == 0), stop=(k == 7))
            nc.tensor.matmul(psQ1, lhsT=wq_sb[:, k, :],
                             rhs=xT_sb[:, k, 512:1024],
                             start=(k == 0), stop=(k == 7))
        nc.vector.tensor_scalar_add(QT[:, 0:512], psQ0, bq_sb)
        nc.vector.tensor_scalar_add(QT[:, 512:1024], psQ1, bq_sb)

        # deferred PE work, interleaved into the ACT-bound attention loop.
        # entry = (step, thunk): emitted at the given tt step of that chunk.
        # Budget: each chunk step has ~1.0us of PE slack beside its own
        # scores pair + the previous chunk's PV; chunk 0 has no PV so it
        # absorbs the V-batch0 build and most of the KT back-fill.
        qk = [(wq_sb, bq_sb, QT), (wk_sb, bk_sb, KT)]

        def pj(which, n256):
            return lambda: emit_proj_chunk(*qk[which], n256, w=256)

        def spread(thunks, start, stop):
            n = len(thunks)
            return [(start + (i * (stop - start)) // n, t)
                    for i, t in enumerate(thunks)]

        def v_ex(cs, start, stop):
            # V proj chunks cs (512-token units) + their transposes
            ex = []
            n = len(cs)
            for i, c in enumerate(cs):
                st = start + (i * (stop - start)) // n
                ex.append((st, lambda c=c: emit_v_proj(c)))
                ex += [(st + 1 + j, lambda t=4 * c + j: emit_v_tr(t))
                       for j in range(4)]
            return ex

        extras_per_chunk = [
            # chunk 0 (b0,sc0; no PV, ~23us slack): KT units 1-10 just
            # ahead of use (unit u feeds scores step 2u), QT 4-7 (chunk 1
            # queries), V batch0 (tiles 0-15, needed by chunk 1's PV) and
            # the first batch1 V chunk (c4: its x arrives by ~45us)
            [(0, pj(1, 1)), (1, pj(1, 2)), (2, pj(1, 3)), (3, pj(1, 4)),
             (5, pj(1, 5)), (7, pj(1, 6)), (9, pj(1, 7)), (11, pj(1, 8)),
             (13, pj(1, 9)), (14, pj(1, 10))]
            + spread([pj(0, n) for n in (4, 5, 6, 7)], 8, 13)
            + v_ex([0, 1, 2], 1, 7) + v_ex([3, 4], 7, 12),
            # chunk 1 (b0,sc1): QT 8-11 (chunk 2 queries), KT 11-13, V c5
            spread([pj(0, n) for n in (8, 9, 10, 11)], 0, 8)
            + [(8, pj(1, 11)), (10, pj(1, 12)), (12, pj(1, 13))]
            + v_ex([5], 4, 9),
            # chunk 2 (b1,sc0): KT 14-15 (feed scores steps 12/14), QT
            # 12-15 (chunk 3 queries), V batch1 rest (tiles 24-31)
            [(0, pj(1, 14)), (2, pj(1, 15))]
            + spread([pj(0, n) for n in (12, 13, 14, 15)], 3, 10)
            + v_ex([6], 5, 10) + v_ex([7], 9, 14),
            # chunk 3 (b1,sc1): WO tiles 0-15 (tokens of chunks 0-1, all
            # normalized by the end of chunk 2)
            [((i * 14) // 16, lambda t=t: emit_wo_tile(t))
             for i, t in enumerate(range(16))],
        ]

        def emit_scores(b, sc, tt):
            # one t-tile of S^T for both heads -> psum pair; returns the pair
            s0 = b * S + sc * SC
            pair = []
            for h in range(HPC):
                ps = psp.tile([128, SC], F32, tag="ps")
                hsl = slice(h * DK, (h + 1) * DK)
                for n2 in range(SC // 512):
                    nc.tensor.matmul(
                        ps[:, ts(n2, 512)],
                        lhsT=KT[hsl, b * S + tt * 128:b * S + (tt + 1) * 128],
                        rhs=QT[hsl, s0 + n2 * 512:s0 + (n2 + 1) * 512],
                        start=True, stop=True)
                pair.append(ps)
            return pair

        chunks = [(b, sc) for b in range(B) for sc in range(S // SC)]
        prev = None
        next_pair = None
        for ci, (b, sc) in enumerate(chunks):
            extras = sorted(extras_per_chunk[ci], key=lambda e: e[0])
            pts = []
            cur = (b, sc, pts, {})
            pair = next_pair if next_pair is not None else emit_scores(b, sc, 0)
            for tt in range(16):
                row = []
                for h in range(HPC):
                    pt = ptp.tile([128, SC], BF16, tag="pt")
                    nc.scalar.activation(
                        out=pt, in_=pair[h],
                        func=mybir.ActivationFunctionType.Exp,
                        scale=0.125)
                    row.append(pt)
                pts.append(row)
                # emit next scores ahead of the slower PE work so ScalarE's
                # psum slots refill as soon as its exp frees them
                if tt + 1 < 16:
                    pair = emit_scores(b, sc, tt + 1)
                if prev is not None:
                    emit_pv_step(prev, tt)
                while extras and extras[0][0] <= tt:
                    extras.pop(0)[1]()
            # next chunk's first scores go ahead of the extras flush so the
            # boundary never starves ScalarE
            if ci + 1 < len(chunks):
                nb, nsc = chunks[ci + 1]
                next_pair = emit_scores(nb, nsc, 0)
            for _, e in extras:
                e()
            prev = cur
        # tail: PV + normalize for the last chunk with WO 16-23 (ready at
        # tail start) at odd steps.  WO 24-31 wait on the tail's own
        # normalizes, so they run strictly after the PV loop — putting any
        # of them mid-loop would stall the in-order PE queue on the
        # normalize chain and block the remaining PV steps behind it.
        for s in range(16):
            emit_pv_step(prev, s)
            if s % 2 == 1:
                emit_wo_tile(16 + s // 2, use_act=True)
        for tt in range(24, 32):
            emit_wo_tile(tt, use_act=True)


def _prep_in_maps(x, wq, bq, wk, bk, wv, bv, wo):
    x2 = np.asarray(x, np.float32).reshape(NT, D)
    xT = np.ascontiguousarray(x2.T).astype(NPBF16)
    wq = np.asarray(wq, np.float32)
    wk = np.asarray(wk, np.float32)
    wv = np.asarray(wv, np.float32)
    wo = np.asarray(wo, np.float32)
    bq = np.asarray(bq, np.float32)
    bk = np.asarray(bk, np.float32)
    bv = np.asarray(bv, np.float32)

    def pkc(w, cs):
        # [1024, 128-cols-of-core] -> [p=128, k=8, c=128] contiguous
        wc = w[:, cs].reshape(8, 128, 128).transpose(1, 0, 2)
        return np.ascontiguousarray(wc).astype(NPBF16)

    in_maps = []
    for c in range(NCORES):
        cs = slice(c * 128, (c + 1) * 128)
        in_maps.append({
            "xT": xT,
            "wq": pkc(wq, cs),
            "wk": pkc(wk, cs),
            "wv": pkc(wv, cs),
            "bqkv": np.ascontiguousarray(
                np.stack([bq[cs], bk[cs], bv[cs]], axis=1)),
            "wo": wo[cs, :].astype(NPBF16),
        })
    return in_maps


def kernel(x, wq, bq, wk, bk, wv, bv, wo, bo, _run_kwargs=None):
    if "nc" not in _CACHE:
        _CACHE["nc"] = _build_nc()
    nc = _CACHE["nc"]
    in_maps = _prep_in_maps(x, wq, bq, wk, bk, wv, bv, wo)
    res = run_bass_kernel_spmd(nc, in_maps, list(range(NCORES)),
                               **(_run_kwargs or {}))
    acc = np.zeros((NT, D), np.float32)
    for c in range(NCORES):
        acc += res.results[c]["out"].astype(np.float32)
    acc += np.asarray(bo, np.float32)[None, :]
    if _run_kwargs:
        _CACHE["last_results"] = res
    return acc.reshape(B, S, D)
